# revision 1
# baseline (speedup 1.0000x reference)
"""Multi-head attention TRN2 kernel (nn_MultiHeadAttention_69922067579127).

Full-input contract: kernel(**inputs) takes the complete tensors and
returns the complete output. Internally: tensor-parallel over heads —
each of the 8 NeuronCores computes 2 of the 16 heads (QKV projection,
attention, and its slice of the output projection); the 8 partial
outputs are summed on the host (the output projection is linear in the
per-head contributions) and b_out is added once.

All matmuls run in float32r (TRN2's fast fp32 PE mode, ~1.5e-4 rel err)
with fp32 accumulation in PSUM; elementwise math is fp32.

Layout strategy per core (heads h0, h1):
  - x [8192, 1024] is transposed on-chip (PE transpose via identity) in
    512-token chunks to feed QKV as [feat, tok].
  - QKV^T [384, tok] = W_slice.T @ x^T; rows = [q(128) | k(128) | v(128)],
    each 128 = h0's 64 dims then h1's 64 dims. + bias (per-partition).
  - scores^T [k_tok, q_tok]: per kt the two heads' K=64 matmuls run
    concurrently on PE row-groups 0-63 / 64-127 into halves of one
    [128, 1024] PSUM tile; one [128, 1024] exp per kt on ScalarE.
  - softmax without max-subtraction (inputs are N(0,1)-scale gaussians;
    scores ~ N(0,1), exp is safe in fp32), with the 1/8 scale folded
    into the exp; denominators come from ones-rows appended to V in the
    AV matmul (output rows 64/65).
  - AV: out^T [66, q] = [V | 1 | 1]-layout lhsT (token-major V, made by
    PE-transposing V^T) against exp(S^T), accumulated over kt in PSUM.
  - normalization: raw denominator row is broadcast across 64
    partitions via a K=1 PE outer product, approx-reciprocal'd on
    VectorE (2-op Newton-Raphson, ~2 ULP), and multiplied into the
    attention output; h1's rows are moved to partitions 64-127 with a
    SBUF->SBUF DMA so the output projection runs as one K=128 matmul.
  - the attention kt-loop is software-pipelined one kt deep against the
    exp, and the next batch's transpose/QKV work is emitted between kt
    steps (generator interleave) to fill PE bubbles; qT/kT/vtok are
    double-buffered across batches.
"""

import sys

sys.path.insert(0, "/opt/trn_rl_repo")

from contextlib import ExitStack

import numpy as np

import concourse.bacc as bacc
import concourse.mybir as mybir
import concourse.tile as tile
from concourse.bass_utils import run_bass_kernel_spmd
from concourse.masks import make_identity

F32 = mybir.dt.float32
F32R = mybir.dt.float32r
EXP = mybir.ActivationFunctionType.Exp

B, T, D = 4, 2048, 1024
H, Dh = 16, 64
BT = B * T            # 8192 tokens
NCORES = 8
HPC = H // NCORES     # 2 heads per core
QC = 256              # query-chunk (columns of S^T per block)
NQC = T // QC         # 8 per batch
KT = T // 128         # 16 key-token tiles per batch
TC = 512              # token chunk for x transpose + QKV
NTC = T // TC         # 4 per batch

_CACHE = {}


def _build():
    nc = bacc.Bacc("TRN2", target_bir_lowering=False, debug=False)
    x = nc.dram_tensor("x", [BT, D], F32, kind="ExternalInput").ap()
    wqkv = nc.dram_tensor("wqkv", [D, 3 * 128], F32, kind="ExternalInput").ap()
    bqkv = nc.dram_tensor("bqkv", [3 * 128], F32, kind="ExternalInput").ap()
    wout = nc.dram_tensor("wout", [128, D], F32, kind="ExternalInput").ap()
    out = nc.dram_tensor("out", [BT, D], F32, kind="ExternalOutput").ap()

    with tile.TileContext(nc) as tc, ExitStack() as ctx:
        const = ctx.enter_context(tc.tile_pool(name="const", bufs=1))
        perb = ctx.enter_context(tc.tile_pool(name="perb", bufs=1))
        xsp = ctx.enter_context(tc.tile_pool(name="xsp", bufs=2))
        xtp = ctx.enter_context(tc.tile_pool(name="xtp", bufs=2))
        stp = ctx.enter_context(tc.tile_pool(name="stp", bufs=4))
        work = ctx.enter_context(tc.tile_pool(name="work", bufs=1))
        outp = ctx.enter_context(tc.tile_pool(name="outp", bufs=3))
        # PSUM: 8 banks total. "mm" 2x1 + "sc" 2x2 + "av" 2x1 = 8.
        psA = ctx.enter_context(tc.tile_pool(name="psA", bufs=2, space="PSUM"))
        pssc = ctx.enter_context(tc.tile_pool(name="pssc", bufs=2, space="PSUM"))
        psav = ctx.enter_context(tc.tile_pool(name="psav", bufs=2, space="PSUM"))

        # ---- constants ----
        ident = const.tile([128, 128], F32)
        make_identity(nc, ident)

        ones_f = const.tile([128, 64], F32)
        nc.vector.memset(ones_f, 1.0)
        ones_r = const.tile([128, 64], F32R)
        nc.vector.tensor_copy(out=ones_r, in_=ones_f)

        w_f = xsp.tile([128, 8, 384], F32, tag="xs")
        nc.sync.dma_start(out=w_f, in_=wqkv.rearrange("(ko ki) m -> ki ko m", ki=128))
        w_r = const.tile([128, 8, 384], F32R)
        nc.vector.tensor_copy(out=w_r, in_=w_f)

        bq_sb = const.tile([128, 3], F32)
        nc.sync.dma_start(out=bq_sb, in_=bqkv.rearrange("(m p) -> p m", p=128))

        wo_f = xsp.tile([128, D], F32, tag="xs")
        nc.sync.dma_start(out=wo_f, in_=wout)
        wo_r = const.tile([128, D], F32R)
        nc.vector.tensor_copy(out=wo_r, in_=wo_f)

        # ---- persistent tiles ----
        vTt = perb.tile([128, T], F32)     # V^T, pre-transpose
        attnT = perb.tile([128, T], F32R)  # normalized attn out (both heads)
        perb2 = ctx.enter_context(tc.tile_pool(name="perb2", bufs=2))
        ones4 = ones_f.rearrange("p (k h c) -> p k h c", h=2, c=2)

        tiles = {}

        def start_b(bb):
            qT_b = perb2.tile([128, T], F32R, tag="qT", name="qT")
            kT_b = perb2.tile([128, T], F32R, tag="kT", name="kT")
            # token-major V per key-tile: per head 66 cols = [v(64) | 1 | 1]
            vtok_b = perb2.tile([128, KT, 2 * 66], F32R, tag="vtok", name="vtok")
            nc.vector.tensor_copy(
                out=vtok_b.rearrange("p k (h c) -> p k h c", c=66)[:, :, :, 64:66],
                in_=ones4,
            )
            tiles[bb] = (qT_b, kT_b, vtok_b)

        def phase_a_chunk(bb, tci):
            """Generator: x^T + QKV^T + V token-major for one 512-token
            chunk. Yields at op-group boundaries so the caller can
            interleave these PE ops into attention's exp-wait bubbles."""
            qT_b, kT_b, vtok_b = tiles[bb]
            r0 = bb * T + tci * TC
            xs = xsp.tile([128, TC // 128, D], F32, tag="xs", name="xs")
            nc.sync.dma_start(
                out=xs, in_=x[r0 : r0 + TC, :].rearrange("(tt p) f -> p tt f", p=128)
            )
            yield
            xt = xtp.tile([128, 8, TC], F32R, tag="xt", name="xt")
            for tt in range(TC // 128):
                for fo in range(8):
                    pst = psA.tile([128, 128], F32, tag="mm", name="pst")
                    nc.tensor.transpose(
                        pst, xs[:, tt, fo * 128 : (fo + 1) * 128], ident
                    )
                    nc.vector.tensor_copy(
                        out=xt[:, fo, tt * 128 : (tt + 1) * 128], in_=pst
                    )
                    if fo % 2 == 1:
                        yield
            for m in range(3):
                psq = psA.tile([128, TC], F32, tag="mm", name="psq")
                for ko in range(8):
                    nc.tensor.matmul(
                        psq,
                        w_r[:, ko, m * 128 : (m + 1) * 128],
                        xt[:, ko, :],
                        start=(ko == 0),
                        stop=(ko == 7),
                    )
                    if ko == 3:
                        yield
                dst = (qT_b, kT_b, vTt)[m]
                nc.vector.tensor_scalar_add(
                    out=dst[:, tci * TC : (tci + 1) * TC],
                    in0=psq,
                    scalar1=bq_sb[:, m : m + 1],
                )
                yield
            # V^T -> token-major V for this chunk's 4 key-tiles
            for j in range(4):
                kt = tci * 4 + j
                pst = psA.tile([128, 128], F32, tag="mm", name="pst")
                nc.tensor.transpose(pst, vTt[:, kt * 128 : (kt + 1) * 128], ident)
                nc.vector.tensor_copy(
                    out=vtok_b[:, kt, :].rearrange("p (h c) -> p h c", c=66)[
                        :, :, 0:64
                    ],
                    in_=pst.rearrange("p (h c) -> p h c", c=64),
                )
                yield

        def sweep(bb, sw, filler):
            """One attention q-sweep (512 queries, both heads), with
            phase-A ops for the next batch pulled in between kt steps."""
            qT_b, kT_b, vtok_b = tiles[bb]
            q0 = sw * 512
            av0 = psav.tile([66, 512], F32, tag="av", name="av0")
            av1 = psav.tile([66, 512], F32, tag="av", name="av1")
            sts = [None] * KT

            def _scores(kt):
                sc = pssc.tile([128, 1024], F32, tag="sc", name="sc")
                nc.tensor.matmul(
                    sc[:, 0:512],
                    kT_b[0:64, kt * 128 : (kt + 1) * 128],
                    qT_b[0:64, q0 : q0 + 512],
                    start=True,
                    stop=True,
                )
                nc.tensor.matmul(
                    sc[:, 512:1024],
                    kT_b[64:128, kt * 128 : (kt + 1) * 128],
                    qT_b[64:128, q0 : q0 + 512],
                    start=True,
                    stop=True,
                )
                st = stp.tile([128, 1024], F32R, tag="st", name="st")
                nc.scalar.activation(out=st, in_=sc, func=EXP, scale=0.125)
                sts[kt] = st

            def _av(kt):
                st = sts[kt]
                nc.tensor.matmul(
                    av0,
                    vtok_b[:, kt, 0:66],
                    st[:, 0:512],
                    start=(kt == 0),
                    stop=(kt == KT - 1),
                )
                nc.tensor.matmul(
                    av1,
                    vtok_b[:, kt, 66:132],
                    st[:, 512:1024],
                    start=(kt == 0),
                    stop=(kt == KT - 1),
                )

            _scores(0)
            for kt in range(1, KT):
                _scores(kt)
                _av(kt - 1)
                next(filler, None)
                next(filler, None)
            _av(KT - 1)
            # stage denominator rows (row 64 of each AV psum) as f32r
            drow_r = work.tile([128, 2, 512], F32R, tag="drow", name="drow")
            nc.vector.tensor_copy(out=drow_r[64:65, 0, :], in_=av0[64:65, :])
            nc.vector.tensor_copy(out=drow_r[64:65, 1, :], in_=av1[64:65, :])
            # broadcast raw denom across 64 partitions via K=1 outer
            # product, then fast approx reciprocal on all 64 lanes
            bcs = []
            for h in range(2):
                bc = psA.tile([64, 512], F32, tag="mm", name="bc")
                nc.tensor.matmul(
                    bc,
                    ones_r[64:65, :],
                    drow_r[64:65, h, :],
                    start=True,
                    stop=True,
                )
                rec_sb = work.tile([64, 512], F32, tag=f"rec{h}", name=f"rec{h}")
                scr = work.tile([64, 512], F32, tag="scr", name="scr")
                nc.vector.reciprocal_approx_accurate(out=rec_sb, in_=bc, scratch=scr)
                bcs.append(rec_sb)
            # normalized attnT: h0 direct; h1 via SBUF->SBUF DMA part-shift
            nc.vector.tensor_mul(
                out=attnT[0:64, q0 : q0 + 512], in0=av0[0:64, :], in1=bcs[0]
            )
            tmp1 = work.tile([64, 512], F32R, tag="tmp1", name="tmp1")
            nc.vector.tensor_mul(out=tmp1, in0=av1[0:64, :], in1=bcs[1])
            nc.sync.dma_start(out=attnT[64:128, q0 : q0 + 512], in_=tmp1)

            # output projection for this sweep's 4 q-slices (K=128 merged)
            for si in range(4):
                sl = sw * 4 + si
                outsb = outp.tile([128, D], F32, tag="outsb", name="outsb")
                for n in range(2):
                    po = psA.tile([128, 512], F32, tag="mm", name="po")
                    nc.tensor.matmul(
                        po,
                        attnT[:, sl * 128 : (sl + 1) * 128],
                        wo_r[:, n * 512 : (n + 1) * 512],
                        start=True,
                        stop=True,
                    )
                    nc.vector.tensor_copy(
                        out=outsb[:, n * 512 : (n + 1) * 512], in_=po
                    )
                r0 = bb * T + sl * 128
                nc.sync.dma_start(out=out[r0 : r0 + 128, :], in_=outsb)
                next(filler, None)

        # prologue: batch 0's phase A runs un-interleaved
        start_b(0)
        for t in range(NTC):
            for _ in phase_a_chunk(0, t):
                pass
        for b in range(B):
            for sw in range(4):
                if b + 1 < B:
                    if sw == 0:
                        start_b(b + 1)
                    filler = phase_a_chunk(b + 1, sw)
                else:
                    filler = iter(())
                sweep(b, sw, filler)
                for _ in filler:  # drain any leftover phase-A ops
                    pass
            tiles.pop(b)

    nc.compile()
    return nc


def kernel(x, W_qkv, b_qkv, W_out, b_out):
    x = np.ascontiguousarray(np.asarray(x, dtype=np.float32))
    W_qkv = np.asarray(W_qkv, dtype=np.float32)
    b_qkv = np.asarray(b_qkv, dtype=np.float32)
    W_out = np.asarray(W_out, dtype=np.float32)
    b_out = np.asarray(b_out, dtype=np.float32)

    if "nc" not in _CACHE:
        _CACHE["nc"] = _build()
    nc = _CACHE["nc"]

    xf = x.reshape(BT, D)
    in_maps = []
    for c in range(NCORES):
        lo, hi = c * 128, (c + 1) * 128
        wq = np.ascontiguousarray(
            np.concatenate(
                [
                    W_qkv[:, lo:hi],
                    W_qkv[:, D + lo : D + hi],
                    W_qkv[:, 2 * D + lo : 2 * D + hi],
                ],
                axis=1,
            )
        )
        bq = np.ascontiguousarray(
            np.concatenate(
                [b_qkv[lo:hi], b_qkv[D + lo : D + hi], b_qkv[2 * D + lo : 2 * D + hi]]
            )
        )
        wo = np.ascontiguousarray(W_out[lo:hi, :])
        in_maps.append({"x": xf, "wqkv": wq, "bqkv": bq, "wout": wo})

    res = run_bass_kernel_spmd(nc, in_maps, core_ids=list(range(NCORES)))
    acc = np.zeros((BT, D), dtype=np.float64)
    for c in range(NCORES):
        acc += res.results[c]["out"]
    acc += b_out
    return acc.reshape(B, T, D).astype(np.float32)



# revision 4
# speedup vs baseline: 1.6060x; 1.6060x over previous
"""Multi-head attention TRN2 kernel (nn_MultiHeadAttention_69922067579127).

Full-input contract: kernel(**inputs) takes the complete tensors and
returns the complete output. Internally: tensor-parallel over heads --
each of the 8 NeuronCores computes 2 of the 16 heads (QKV projection,
attention, and its slice of the output projection); the 8 partial
outputs are summed on the host (the output projection is linear in the
per-head contributions) and b_out is added once.

v2 design (fp16 + DMA-XBAR transposes, PE runs only real matmuls):
  - x and the weights are cast to fp16 on the host. x^T tiles are
    loaded straight from DRAM with the DMA crossbar transpose
    (dma_start_transpose, 2-byte dtypes), so the PE does ZERO
    transpose-mode ops -- transpose-mode doesn't count as PE-busy for
    the HAM clock gate and was keeping the baseline at 1.2 GHz.
  - QKV^T [384, tok] = W.T @ x^T in fp16 (FWL weight loads), fp32 PSUM,
    bias added on DVE with fp16 output.
  - scores^T [k_tok, q_tok]: per kt the two heads' K=64 matmuls are
    emitted back-to-back targeting PE row groups 0-63 / 64-127 so the
    hardware runs them concurrently (no filler ops between them).
  - softmax without max-subtraction (scores ~ N(0,1)); exp on ScalarE
    with the 1/8 scale folded in, fp16 output; denominators come from
    two ones-columns appended to V in the AV matmul (output rows 64/65).
  - V is re-laid token-major via SBUF->SBUF DMA transposes (not PE).
  - AV accumulates over kt in PSUM; normalization via K=1 broadcast
    matmul of the raw denominator + DVE Newton-Raphson reciprocal;
    h1's rows move to partitions 64-127 with a SBUF->SBUF DMA so the
    output projection runs as one K=128 fp16 matmul.
  - the attention kt-loop is software-pipelined one kt deep against the
    exp, and the next batch's QKV work is emitted between kt steps
    (generator interleave) to fill PE bubbles without ever letting the
    PE idle long enough to re-throttle.
"""

import sys

sys.path.insert(0, "/opt/trn_rl_repo")

from contextlib import ExitStack

import numpy as np

import concourse.bacc as bacc
import concourse.mybir as mybir
import concourse.tile as tile
from concourse.bass_utils import run_bass_kernel_spmd

F16 = mybir.dt.float16
F32 = mybir.dt.float32
F32R = mybir.dt.float32r
EXP = mybir.ActivationFunctionType.Exp

B, T, D = 4, 2048, 1024
H, Dh = 16, 64
BT = B * T            # 8192 tokens
NCORES = 8
HPC = H // NCORES     # 2 heads per core
QC = 512              # query-sweep width (columns of S^T per block)
KT = T // 128         # 16 key-token tiles per batch
TC = 512              # token chunk for QKV
NTC = T // TC         # 4 per batch

_CACHE = {}


def _build():
    nc = bacc.Bacc("TRN2", target_bir_lowering=False, debug=False)
    x = nc.dram_tensor("x", [BT, D], F16, kind="ExternalInput").ap()
    wqkv = nc.dram_tensor("wqkv", [D, 3 * 128], F16, kind="ExternalInput").ap()
    bqkv = nc.dram_tensor("bqkv", [3 * 128], F32, kind="ExternalInput").ap()
    wout = nc.dram_tensor("wout", [128, D], F16, kind="ExternalInput").ap()
    out = nc.dram_tensor("out", [BT, D], F32, kind="ExternalOutput").ap()

    with tile.TileContext(nc) as tc, ExitStack() as ctx:
        const = ctx.enter_context(tc.tile_pool(name="const", bufs=1))
        perb = ctx.enter_context(tc.tile_pool(name="perb", bufs=1))
        perb2 = ctx.enter_context(tc.tile_pool(name="perb2", bufs=2))
        stp = ctx.enter_context(tc.tile_pool(name="stp", bufs=4))
        work = ctx.enter_context(tc.tile_pool(name="work", bufs=1))
        outp = ctx.enter_context(tc.tile_pool(name="outp", bufs=3))
        # PSUM: 8 banks total. "mm" 2x1 + "sc" 2x2 + "av" 2x1 = 8.
        psA = ctx.enter_context(tc.tile_pool(name="psA", bufs=2, space="PSUM"))
        pssc = ctx.enter_context(tc.tile_pool(name="pssc", bufs=2, space="PSUM"))
        psav = ctx.enter_context(tc.tile_pool(name="psav", bufs=2, space="PSUM"))

        # ---- constants ----
        ones_f = const.tile([128, 64], F32)
        nc.vector.memset(ones_f, 1.0)
        ones_h = const.tile([128, 64], F16)
        nc.vector.tensor_copy(out=ones_h, in_=ones_f)
        ones4 = ones_h.rearrange("p (k h c) -> p k h c", h=2, c=2)

        ones_r = const.tile([128, 64], F32R)
        nc.vector.tensor_copy(out=ones_r, in_=ones_f)

        w_r = const.tile([128, 8, 384], F16)
        nc.sync.dma_start(out=w_r, in_=wqkv.rearrange("(ko ki) m -> ki ko m", ki=128))

        bq_sb = const.tile([128, 3], F32)
        nc.sync.dma_start(out=bq_sb, in_=bqkv.rearrange("(m p) -> p m", p=128))

        wo_r = const.tile([128, D], F16)
        nc.sync.dma_start(out=wo_r, in_=wout)

        # ---- persistent tiles ----
        vTt = perb.tile([128, T], F16)     # V^T (feature-major), per batch
        attnT = perb.tile([128, T], F16)   # normalized attn out (both heads)

        tiles = {}

        def start_b(bb):
            qT_b = perb2.tile([128, T], F16, tag="qT", name="qT")
            kT_b = perb2.tile([128, T], F16, tag="kT", name="kT")
            # token-major V per key-tile: per head 80 cols = [v(64) | 1 | 1 | pad]
            # (pad to 80 so the XBAR transpose destination blocks are
            # 32B-aligned -- misaligned strided writes corrupt neighbors)
            vtok_b = perb2.tile([128, KT, 2, 80], F16, tag="vtok", name="vtok")
            xt_b = perb2.tile([128, 8, T], F16, tag="xt", name="xt")
            nc.vector.tensor_copy(out=vtok_b[:, :, :, 64:66], in_=ones4)
            # x^T for the whole batch via DMA crossbar transpose:
            # xt[p, ko, t] = x[bb*T + t, ko*128 + p]
            for tci in range(NTC):
                r0 = bb * T + tci * TC
                nc.sync.dma_start_transpose(
                    out=xt_b[:, :, tci * TC : (tci + 1) * TC],
                    in_=x[r0 : r0 + TC, :],
                )
            tiles[bb] = (qT_b, kT_b, vtok_b, xt_b)

        def phase_a_chunk(bb, tci):
            """Generator: QKV^T + token-major V for one 512-token chunk.
            Yields at ~2-matmul boundaries so the caller can interleave
            these PE ops into attention's exp-wait bubbles."""
            qT_b, kT_b, vtok_b, xt_b = tiles[bb]
            for m in range(3):
                psq = psA.tile([128, TC], F32, tag="mm", name="psq")
                for ko in range(8):
                    nc.tensor.matmul(
                        psq,
                        w_r[:, ko, m * 128 : (m + 1) * 128],
                        xt_b[:, ko, tci * TC : (tci + 1) * TC],
                        start=(ko == 0),
                        stop=(ko == 7),
                    )
                    if ko % 2 == 1:
                        yield
                dst = (qT_b, kT_b, vTt)[m]
                nc.vector.tensor_scalar_add(
                    out=dst[:, tci * TC : (tci + 1) * TC],
                    in0=psq,
                    scalar1=bq_sb[:, m : m + 1],
                )
                yield
            # V^T -> token-major V via SBUF->SBUF DMA transpose (one per
            # head): vtok[p, tci*4+j, h, f] = vTt[h*64+f, tci*512+j*128+p]
            for h in range(2):
                nc.sync.dma_start_transpose(
                    out=vtok_b[:, tci * 4 : (tci + 1) * 4, h, 0:64],
                    in_=vTt[h * 64 : (h + 1) * 64, tci * TC : (tci + 1) * TC],
                )
            yield

        def sweep(bb, sw, filler):
            """One attention q-sweep (512 queries, both heads), with
            phase-A ops for the next batch pulled in between kt steps."""
            qT_b, kT_b, vtok_b, xt_b = tiles[bb]
            q0 = sw * QC
            av0 = psav.tile([66, QC], F32, tag="av", name="av0")
            av1 = psav.tile([66, QC], F32, tag="av", name="av1")
            sts = [None] * KT

            def _scores(kt):
                sc = pssc.tile([128, 1024], F32, tag="sc", name="sc")
                # two heads back-to-back on disjoint PE row groups ->
                # hardware runs them concurrently
                nc.tensor.matmul(
                    sc[:, 0:512],
                    kT_b[0:64, kt * 128 : (kt + 1) * 128],
                    qT_b[0:64, q0 : q0 + QC],
                    start=True,
                    stop=True,
                )
                nc.tensor.matmul(
                    sc[:, 512:1024],
                    kT_b[64:128, kt * 128 : (kt + 1) * 128],
                    qT_b[64:128, q0 : q0 + QC],
                    start=True,
                    stop=True,
                )
                st = stp.tile([128, 1024], F16, tag="st", name="st")
                nc.scalar.activation(out=st, in_=sc, func=EXP, scale=0.125)
                sts[kt] = st

            def _av(kt):
                st = sts[kt]
                nc.tensor.matmul(
                    av0,
                    vtok_b[:, kt, 0, 0:66],
                    st[:, 0:512],
                    start=(kt == 0),
                    stop=(kt == KT - 1),
                )
                nc.tensor.matmul(
                    av1,
                    vtok_b[:, kt, 1, 0:66],
                    st[:, 512:1024],
                    start=(kt == 0),
                    stop=(kt == KT - 1),
                )

            _scores(0)
            for kt in range(1, KT):
                _scores(kt)
                _av(kt - 1)
                next(filler, None)
                next(filler, None)
            _av(KT - 1)
            # stage denominator rows (row 64 of each AV psum) as f32r
            drow_r = work.tile([128, 2, QC], F32R, tag="drow", name="drow")
            nc.vector.tensor_copy(out=drow_r[64:65, 0, :], in_=av0[64:65, :])
            nc.vector.tensor_copy(out=drow_r[64:65, 1, :], in_=av1[64:65, :])
            # broadcast raw denom across 64 partitions via K=1 outer
            # product, then fast approx reciprocal on all 64 lanes
            bcs = []
            for h in range(2):
                bc = psA.tile([64, QC], F32, tag="mm", name="bc")
                nc.tensor.matmul(
                    bc,
                    ones_r[64:65, :],
                    drow_r[64:65, h, :],
                    start=True,
                    stop=True,
                )
                rec_sb = work.tile([64, QC], F32, tag=f"rec{h}", name=f"rec{h}")
                scr = work.tile([64, QC], F32, tag="scr", name="scr")
                nc.vector.reciprocal_approx_accurate(out=rec_sb, in_=bc, scratch=scr)
                bcs.append(rec_sb)
            # normalized attnT: h0 direct; h1 via SBUF->SBUF DMA part-shift
            nc.vector.tensor_mul(
                out=attnT[0:64, q0 : q0 + QC], in0=av0[0:64, :], in1=bcs[0]
            )
            tmp1 = work.tile([64, QC], F16, tag="tmp1", name="tmp1")
            nc.vector.tensor_mul(out=tmp1, in0=av1[0:64, :], in1=bcs[1])
            nc.sync.dma_start(out=attnT[64:128, q0 : q0 + QC], in_=tmp1)

            # output projection for this sweep's 4 q-slices (K=128 merged)
            for si in range(4):
                sl = sw * 4 + si
                outsb = outp.tile([128, D], F32, tag="outsb", name="outsb")
                for n in range(2):
                    po = psA.tile([128, 512], F32, tag="mm", name="po")
                    nc.tensor.matmul(
                        po,
                        attnT[:, sl * 128 : (sl + 1) * 128],
                        wo_r[:, n * 512 : (n + 1) * 512],
                        start=True,
                        stop=True,
                    )
                    nc.vector.tensor_copy(
                        out=outsb[:, n * 512 : (n + 1) * 512], in_=po
                    )
                r0 = bb * T + sl * 128
                nc.sync.dma_start(out=out[r0 : r0 + 128, :], in_=outsb)
                next(filler, None)

        # prologue: batch 0's phase A runs un-interleaved
        start_b(0)
        for t in range(NTC):
            for _ in phase_a_chunk(0, t):
                pass
        for b in range(B):
            for sw in range(4):
                if b + 1 < B:
                    if sw == 0:
                        start_b(b + 1)
                    filler = phase_a_chunk(b + 1, sw)
                else:
                    filler = iter(())
                sweep(b, sw, filler)
                for _ in filler:  # drain any leftover phase-A ops
                    pass
            tiles.pop(b)

    nc.compile()
    return nc


def _in_maps(x, W_qkv, b_qkv, W_out):
    xf = np.ascontiguousarray(
        np.asarray(x, dtype=np.float32).reshape(BT, D)
    ).astype(np.float16)
    W_qkv = np.asarray(W_qkv, dtype=np.float32)
    b_qkv = np.asarray(b_qkv, dtype=np.float32)
    W_out = np.asarray(W_out, dtype=np.float32)
    in_maps = []
    for c in range(NCORES):
        lo, hi = c * 128, (c + 1) * 128
        wq = np.ascontiguousarray(
            np.concatenate(
                [
                    W_qkv[:, lo:hi],
                    W_qkv[:, D + lo : D + hi],
                    W_qkv[:, 2 * D + lo : 2 * D + hi],
                ],
                axis=1,
            )
        ).astype(np.float16)
        bq = np.ascontiguousarray(
            np.concatenate(
                [b_qkv[lo:hi], b_qkv[D + lo : D + hi], b_qkv[2 * D + lo : 2 * D + hi]]
            )
        )
        wo = np.ascontiguousarray(W_out[lo:hi, :]).astype(np.float16)
        in_maps.append({"x": xf, "wqkv": wq, "bqkv": bq, "wout": wo})
    return in_maps


def kernel(x, W_qkv, b_qkv, W_out, b_out):
    b_out = np.asarray(b_out, dtype=np.float32)

    if "nc" not in _CACHE:
        _CACHE["nc"] = _build()
    nc = _CACHE["nc"]

    in_maps = _in_maps(x, W_qkv, b_qkv, W_out)
    res = run_bass_kernel_spmd(nc, in_maps, core_ids=list(range(NCORES)))
    acc = np.zeros((BT, D), dtype=np.float64)
    for c in range(NCORES):
        acc += res.results[c]["out"]
    acc += b_out
    return acc.reshape(B, T, D).astype(np.float32)
